# revision 18
# baseline (speedup 1.0000x reference)
"""CRF negative-log-likelihood loss on 8 Trainium2 NeuronCores (Bass/Tile).

Problem: nn_CRF — logits [2048, 512, 32], y_ent [2048, 512], lens [2048],
transitions [32, 32] -> per-sequence NLL [2048] = logZ - gold_path_score.

Strategy (data parallel over batch, 256 seqs/core, chunked rank-1 logZ):

  The scaled-domain forward recursion u <- W_t * (E^T u) is a product of
  per-step transfer matrices A_t = diag(W_t) E^T.  Split T=512 into C=8
  chunks of L=64 steps; each chunk's product M_c mixes so strongly
  (lambda2/lambda1 ~ 0.3 per step; chunks touching padding are *exactly*
  rank one) that M_c ~= f_c g_c^T / s_c with
      f_c  = M_c x_c        (fwd vector pass,  64 serial steps)
      z0_c : g_c = E z0_c = M_c^T (E y_c)   (bwd gamma pass, 64 steps)
  All 16 chunk-passes run in parallel lanes, so the device scan is 64
  serial steps of *wide* ops instead of 256 steps of narrow ones:
  per step one [128,512] matmul + one [128,512] multiply per direction.
  fwd consumes W slab sigma, bwd consumes slab 63-sigma of the SAME
  resident W stream (zero duplicate HBM traffic); DMA pieces arrive
  ends-first so step 0 is ready after ~1MB.

  The combine (inner products across chunk boundaries, logs, scale
  constants) runs on the host in f64 from the shipped bf16 f/z0 tiles:
      logZ = log(yt_7^T f_7) ... telescoped as
           = sum_c -log(1^T E^T f_c) + sum_c log(g_c . f_{c-1})
             + log(g_0[START]) + HC - 32 ln 2
  (the END-chunk init BOOST 2^32 contributes the 32 ln 2; HC restores the
  per-step rowmax M and calibration constant C, pad steps are exact no-ops
  via the BOOST * 2^-32 trick as before).

  The gold path score is an indexed sum: host gathers the per-step terms,
  the ACT engine reduces them while the scan runs.

Layout per core: lane (chunk c, dir) x seq b; tile column x = c*64 + b%64,
partition p = 32*q + tag with q = b//64.  State tiles [128, 512] bf16,
PSUM [128, 512] f32, W stream [128, 64, 512] bf16 resident in SBUF.
"""

import math
import sys

for _p in ("/opt/trn_rl_repo", "/opt/pypackages"):
    if _p not in sys.path:
        sys.path.append(_p)

import numpy as np
import ml_dtypes

BF16 = ml_dtypes.bfloat16
F32 = np.float32

B, T, K = 2048, 512, 32
NCORES = 8
BS = B // NCORES            # 256 sequences per core
C = 8                       # chunks per sequence
L = T // C                  # 64 serial scan steps
NCOL = 512                  # state-tile columns = C * 64
START_IDX, END_IDX = 0, 1
CLIP = float(32.0 * math.log(2.0))
BOOST = float(2.0 ** 32)
TERMS_F = 1032              # 512 e-terms + 513 t-terms + 7 zero pad

# W DMA pieces (slab ranges): step sigma needs slab sigma (fwd) and
# 63-sigma (bwd).  Issued across three DGE queues in parallel so descriptor
# generation doesn't serialize: SP takes the early fwd slabs, ACT the early
# bwd slabs, Pool (slow SWDGE) the late middle.
PIECES_SP = [(0, 2), (2, 5), (5, 10), (10, 18)]
PIECES_ACT = [(62, 64), (59, 62), (54, 59), (46, 54)]
PIECES_POOL = [(18, 30), (34, 46), (30, 34)]
PIECES = PIECES_SP + PIECES_ACT + PIECES_POOL

# Hadamard column split per direction: NA on DVE direct from PSUM (f32);
# the rest is copied PSUM->SBUF bf16 by ACT, then multiplied by DVE in
# all-SBUF 4x mode (NB cols) and by Pool/GPSIMD (rest).
NA = 280
NB = 160
NCPY = NCOL - NA            # 232 copied columns
NP = NCOL - NA - NB         # 72 Pool columns

TRACE = False               # test.py sets True to capture an NTFF profile
LAST_RESULTS = None         # BassKernelResults of the last run (for test.py)

_CACHE = {}


def _build_program():
    if "nc" in _CACHE:
        return _CACHE["nc"]
    import concourse.bacc as bacc
    import concourse.tile as tile
    from concourse import mybir

    nc = bacc.Bacc("TRN2", target_bir_lowering=False, debug=False,
                   enable_asserts=False)
    bf = mybir.dt.bfloat16
    f32 = mybir.dt.float32

    wdev = nc.dram_tensor("wdev", [128, L, NCOL], bf, kind="ExternalInput")
    # cpack = [lhs_f | lhs_g | initf | initg] in one DMA-able constant
    cpack = nc.dram_tensor("cpack", [128, 256 + 2 * NCOL], bf,
                           kind="ExternalInput")
    terms = nc.dram_tensor("terms", [2, 128, TERMS_F], f32,
                           kind="ExternalInput")
    out_f = nc.dram_tensor("out_f", [128, NCOL], bf, kind="ExternalOutput")
    out_g = nc.dram_tensor("out_g", [128, NCOL], bf, kind="ExternalOutput")
    out_score = nc.dram_tensor("out_score", [2, 128, 1], f32,
                               kind="ExternalOutput")

    with tile.TileContext(nc) as tc:
        with (
            tc.tile_pool(name="const", bufs=1) as constp,
            tc.tile_pool(name="wstream", bufs=1) as wp,
            tc.tile_pool(name="stF", bufs=2) as stFp,
            tc.tile_pool(name="stG", bufs=2) as stGp,
            tc.tile_pool(name="cpF", bufs=2) as cpFp,
            tc.tile_pool(name="cpG", bufs=2) as cpGp,
            tc.tile_pool(name="fin", bufs=1) as finp,
            tc.tile_pool(name="psF", bufs=2, space="PSUM") as psFp,
            tc.tile_pool(name="psG", bufs=2, space="PSUM") as psGp,
        ):
            # scan-critical consts: ONE sync-queue DMA
            cp_t = constp.tile([128, 256 + 2 * NCOL], bf)
            nc.sync.dma_start(out=cp_t[:], in_=cpack[:])
            lhs_f = cp_t[:, 0:128]
            lhs_g = cp_t[:, 128:256]
            inf_t = cp_t[:, 256:256 + NCOL]
            ing_t = cp_t[:, 256 + NCOL:256 + 2 * NCOL]

            # W pieces stay resident (written once, never reused), issued
            # over three DGE queues so descriptor generation parallelizes
            piece_t = {}
            for queue, plist in ((nc.sync, PIECES_SP), (nc.scalar, PIECES_ACT),
                                 (nc.gpsimd, PIECES_POOL)):
                for (a, b) in plist:
                    wt = wp.tile([128, b - a, NCOL], bf, tag=f"wt{a}")
                    queue.dma_start(out=wt[:], in_=wdev[:, a:b, :])
                    piece_t[(a, b)] = wt

            def wslab(s, c0, c1):
                for (a, b) in PIECES:
                    if a <= s < b:
                        return piece_t[(a, b)][:, s - a, c0:c1]
                raise AssertionError(s)

            # terms stream in early (read only after the scan by ACT)
            terms_t = []
            for ch in range(2):
                tt = constp.tile([128, TERMS_F], f32, tag=f"terms{ch}")
                nc.gpsimd.dma_start(out=tt[:], in_=terms[ch, :, :])
                terms_t.append(tt)

            stF, stG = inf_t, ing_t
            mult = mybir.AluOpType.mult
            copyf = mybir.ActivationFunctionType.Copy
            for s in range(L):
                for dir_, (stp_, psp, cpp, st) in enumerate((
                        (stFp, psFp, cpFp, stF), (stGp, psGp, cpGp, stG))):
                    lhs = lhs_f if dir_ == 0 else lhs_g
                    slab = s if dir_ == 0 else L - 1 - s
                    v = psp.tile([128, NCOL], f32, tag="v")
                    nc.tensor.matmul(out=v[:], lhsT=lhs, rhs=st[:],
                                     start=True, stop=True)
                    n_ = stp_.tile([128, NCOL], bf, tag="n")
                    # path A: DVE direct from PSUM (f32, 1 elem/cycle)
                    nc.vector.tensor_tensor(out=n_[:, 0:NA], in0=v[:, 0:NA],
                                            in1=wslab(slab, 0, NA), op=mult)
                    # path B: ACT copies PSUM->SBUF bf16; DVE (4x all-SBUF
                    # mode) and Pool multiply from the copy
                    cpy = cpp.tile([128, NCPY], bf, tag="cpy")
                    nc.scalar.activation(out=cpy[:], in_=v[:, NA:NCOL],
                                         func=copyf)
                    nc.vector.tensor_tensor(out=n_[:, NA:NA + NB],
                                            in0=cpy[:, 0:NB],
                                            in1=wslab(slab, NA, NA + NB),
                                            op=mult)
                    nc.gpsimd.tensor_tensor(out=n_[:, NA + NB:NCOL],
                                            in0=cpy[:, NB:NCPY],
                                            in1=wslab(slab, NA + NB, NCOL),
                                            op=mult)
                    if dir_ == 0:
                        stF = n_
                    else:
                        stG = n_

            nc.sync.dma_start(out=out_f[:], in_=stF[:])
            nc.sync.dma_start(out=out_g[:], in_=stG[:])

            # gold-path score: ACT reduces the terms after its scan copies
            dump = constp.tile([128, TERMS_F], f32, tag="dump")
            for ch in range(2):
                sc = finp.tile([128, 1], f32, tag=f"sc{ch}")
                nc.scalar.activation(out=dump[:], in_=terms_t[ch][:],
                                     func=copyf, accum_out=sc[:])
                nc.scalar.dma_start(out=out_score[ch, :, :], in_=sc[:])

    nc.compile()
    _CACHE["nc"] = nc
    return nc


def _calibrate_C(logits, lens_, M, E):
    """Mean per-step growth of the scaled forward recursion, estimated on a
    small subsample.  C only conditions dynamic range, never correctness."""
    bs = np.arange(0, B, max(1, B // 128))
    E64 = E.astype(np.float64)
    lg = logits[bs].astype(np.float64)
    Ms = M[bs].astype(np.float64)
    lv = lens_[bs]
    up = np.zeros((K, len(bs))); up[START_IDX] = 1.0
    grs = []
    for t in range(T // 2):
        up = (E64.T @ up) * np.exp(lg[:, t, :] - Ms[:, t, None]).T
        m = up.max(axis=0)
        live = t < lv
        if live.any():
            grs.append(np.log(m[live]))
        up /= m
        up[:, ~live] = 0.0
        up[START_IDX, ~live] = 1.0
    return float(np.concatenate(grs).mean())


def kernel(logits, y_ent, lens, transitions):
    logits = np.ascontiguousarray(np.asarray(logits), dtype=F32)
    y = np.asarray(y_ent).astype(np.int64)
    lens_ = np.asarray(lens).astype(np.int64)
    trans = np.asarray(transitions).astype(F32)
    assert logits.shape == (B, T, K)

    # ---------------- host preprocessing ----------------
    Tc = np.maximum(trans, F32(-CLIP))
    E = np.exp(Tc.astype(np.float64)).astype(F32)
    E_bf = E.astype(BF16)
    M = logits.max(axis=2)                      # [B, T]
    Cconst = _calibrate_C(logits, lens_, M, E)

    # scaled emissions W[t, j, b] in bf16 with the pad/BOOST trick
    Wb = np.empty((T, K, B), dtype=BF16)
    pad_TB = np.arange(T)[:, None] >= lens_[None, :]          # [T, B]
    for t0 in range(0, T, 32):
        te = t0 + 32
        w = np.exp(logits[:, t0:te, :] - M[:, t0:te, None] - F32(Cconst))
        w = w.transpose(1, 2, 0)                              # [32, K, B]
        pm = pad_TB[t0:te]
        w = np.where(pm[:, None, :], F32(0.0), w)
        w[:, END_IDX, :] = np.where(pm, F32(BOOST), w[:, END_IDX, :])
        Wb[t0:te] = w.astype(BF16)

    # pack per-core W stream: slab[s][32q+j][c*64+bcol] =
    #   W[t=c*L+s, j, b=core*256+q*64+bcol]
    A = Wb.reshape(C, L, K, NCORES, 4, 64)
    A = np.ascontiguousarray(A.transpose(3, 4, 2, 1, 0, 5))
    wdev_np = A.reshape(NCORES, 128, L, NCOL)

    # cpack: cols 0:128 blockdiag(E) x4 (fwd lhsT: out = E^T u), 128:256
    # blockdiag(E^T) x4 (bwd), then initf / initg state tiles.
    # init states: fwd chunk0 = e_START, else ones; bwd chunk7 = BOOST*e_END,
    # else ones  (col x = c*64+bcol, partition 32q+j)
    cpack_np = np.zeros((128, 256 + 2 * NCOL), dtype=BF16)
    for q in range(4):
        cpack_np[32 * q:32 * q + 32, 32 * q:32 * q + 32] = E_bf
        cpack_np[32 * q:32 * q + 32, 128 + 32 * q:128 + 32 * q + 32] = E_bf.T
    initf_np = np.ones((128, NCOL), dtype=BF16)
    initg_np = np.ones((128, NCOL), dtype=BF16)
    colc = np.arange(NCOL) // 64                 # chunk of each column
    pj = np.arange(128) % 32                     # tag of each partition
    initf_np[:, colc == 0] = 0.0
    initf_np[np.ix_(pj == START_IDX, colc == 0)] = 1.0
    initg_np[:, colc == C - 1] = 0.0
    initg_np[np.ix_(pj == END_IDX, colc == C - 1)] = BF16(BOOST)
    cpack_np[:, 256:256 + NCOL] = initf_np
    cpack_np[:, 256 + NCOL:] = initg_np

    # gold-path score terms (host gathers + masks; device sums)
    e_scr = np.take_along_axis(logits, y[:, :, None], axis=2)[:, :, 0]
    e_terms = np.where(np.arange(T)[None, :] < lens_[:, None],
                       e_scr, F32(0.0)).astype(F32)            # [B, 512]
    labels_ext = np.concatenate(
        [np.full((B, 1), START_IDX, np.int64), y,
         np.full((B, 1), END_IDX, np.int64)], axis=1)
    pos = np.arange(T + 2)[None, :]
    labels_ext = np.where(pos < (lens_ + 1)[:, None], labels_ext, END_IDX)
    trn_scr = trans[labels_ext[:, :-1], labels_ext[:, 1:]]
    t_terms = np.where(np.arange(T + 1)[None, :] < (lens_ + 1)[:, None],
                       trn_scr, F32(0.0)).astype(F32)          # [B, 513]
    terms_np = np.zeros((NCORES, 2, 128, TERMS_F), dtype=F32)
    terms_np[..., 0:T] = e_terms.reshape(NCORES, 2, 128, T)
    terms_np[..., T:2 * T + 1] = t_terms.reshape(NCORES, 2, 128, T + 1)

    # per-sequence constant: sum_{t<len}(M + C)
    emask = np.arange(T)[None, :] < lens_[:, None]
    HC = ((M.astype(np.float64) * emask).sum(axis=1) + Cconst * lens_)

    # ---------------- run on the 8 cores ----------------
    nc = _build_program()
    from concourse.bass_utils import run_bass_kernel_spmd

    in_maps = [
        dict(wdev=wdev_np[core], cpack=cpack_np, terms=terms_np[core])
        for core in range(NCORES)
    ]
    res = run_bass_kernel_spmd(nc, in_maps, core_ids=list(range(NCORES)),
                               trace=TRACE)
    global LAST_RESULTS
    LAST_RESULTS = res

    # ---------------- host combine (f64) ----------------
    E64 = E_bf.astype(np.float64)
    logZ = np.zeros(B, np.float64)
    score = np.zeros(B, np.float64)
    for core in range(NCORES):
        r = res.results[core]
        ftile = r["out_f"].astype(np.float64)      # [128, 512]
        gtile = r["out_g"].astype(np.float64)
        sc = r["out_score"].reshape(-1).astype(np.float64)   # [256]
        # unpack [128=(q,j), 512=(c,bcol)] -> [C, K, 256=(q,bcol)]
        f = ftile.reshape(4, K, C, 64).transpose(2, 1, 0, 3).reshape(C, K, BS)
        z0 = gtile.reshape(4, K, C, 64).transpose(2, 1, 0, 3).reshape(C, K, BS)
        lz = np.zeros(BS, np.float64)
        ETf = np.einsum('ij,cib->cjb', E64, f)     # E^T @ f_c
        for c in range(C - 1):
            lz -= np.log(ETf[c].sum(axis=0))       # s_c = ones^T E^T f_c
        g = np.einsum('ij,cjb->cib', E64, z0)      # g_c = E z0_c
        for c in range(1, C):
            lz += np.log((g[c] * f[c - 1]).sum(axis=0))
        lz += np.log(g[0][START_IDX])              # g_0^T u0
        sl = slice(core * BS, (core + 1) * BS)
        logZ[sl] = lz
        score[sl] = sc

    nll = logZ + HC - 32.0 * math.log(2.0) - score
    return nll.astype(F32)


# revision 21
# speedup vs baseline: 1.0705x; 1.0705x over previous
"""CRF negative-log-likelihood loss on 8 Trainium2 NeuronCores (Bass/Tile).

Problem: nn_CRF — logits [2048, 512, 32], y_ent [2048, 512], lens [2048],
transitions [32, 32] -> per-sequence NLL [2048] = logZ - gold_path_score.

Strategy (data parallel over batch, 256 seqs/core, chunked rank-1 logZ):

  The scaled-domain forward recursion u <- W_t * (E^T u) is a product of
  per-step transfer matrices A_t = diag(W_t) E^T.  Split T=512 into C=8
  chunks of L=64 steps; each chunk's product M_c mixes so strongly
  (lambda2/lambda1 ~ 0.3 per step; chunks touching padding are *exactly*
  rank one) that M_c ~= f_c g_c^T / s_c with
      f_c  = M_c x_c        (fwd vector pass,  64 serial steps)
      z0_c : g_c = E z0_c = M_c^T (E y_c)   (bwd gamma pass, 64 steps)
  All 16 chunk-passes run in parallel lanes, so the device scan is 64
  serial steps of *wide* ops instead of 256 steps of narrow ones:
  per step one [128,512] matmul + one [128,512] multiply per direction.
  fwd consumes W slab sigma, bwd consumes slab 63-sigma of the SAME
  resident W stream (zero duplicate HBM traffic); DMA pieces arrive
  ends-first so step 0 is ready after ~1MB.

  The combine (inner products across chunk boundaries, logs, scale
  constants) runs on the host in f64 from the shipped bf16 f/z0 tiles:
      logZ = log(yt_7^T f_7) ... telescoped as
           = sum_c -log(1^T E^T f_c) + sum_c log(g_c . f_{c-1})
             + log(g_0[START]) + HC - 32 ln 2
  (the END-chunk init BOOST 2^32 contributes the 32 ln 2; HC restores the
  per-step rowmax M and calibration constant C, pad steps are exact no-ops
  via the BOOST * 2^-32 trick as before).

  The gold path score is an indexed sum: host gathers the per-step terms,
  the ACT engine reduces them while the scan runs.

Layout per core: lane (chunk c, dir) x seq b; tile column x = c*64 + b%64,
partition p = 32*q + tag with q = b//64.  State tiles [128, 512] bf16,
PSUM [128, 512] f32, W stream [128, 64, 512] bf16 resident in SBUF.
"""

import math
import sys

for _p in ("/opt/trn_rl_repo", "/opt/pypackages"):
    if _p not in sys.path:
        sys.path.append(_p)

import numpy as np
import ml_dtypes

BF16 = ml_dtypes.bfloat16
F32 = np.float32

B, T, K = 2048, 512, 32
NCORES = 8
BS = B // NCORES            # 256 sequences per core
C = 8                       # chunks per sequence
L = T // C                  # 64 serial scan steps
NCOL = 512                  # state-tile columns = C * 64
START_IDX, END_IDX = 0, 1
CLIP = float(32.0 * math.log(2.0))
BOOST = float(2.0 ** 32)
TERMS_F = 1032              # 512 e-terms + 513 t-terms + 7 zero pad

# W DMA pieces (slab ranges): step sigma needs slab sigma (fwd) and
# 63-sigma (bwd).  Issued across three DGE queues in parallel so descriptor
# generation doesn't serialize: SP takes the early fwd slabs, ACT the early
# bwd slabs, Pool (slow SWDGE) the late middle.
PIECES_SP = [(0, 2), (2, 5), (5, 10), (10, 18)]
PIECES_ACT = [(62, 64), (59, 62), (54, 59), (46, 54)]
PIECES_POOL = [(18, 30), (34, 46), (30, 34)]
PIECES = PIECES_SP + PIECES_ACT + PIECES_POOL

# Hadamard column split per direction: NA on DVE direct from PSUM (f32);
# the rest is copied PSUM->SBUF bf16 by ACT, then multiplied by DVE in
# all-SBUF 4x mode (NB cols) and by Pool/GPSIMD (rest, 0 = disabled).
NA = 250
NB = 262
NCPY = NCOL - NA            # copied columns
NP = NCOL - NA - NB         # Pool columns

TRACE = False               # test.py sets True to capture an NTFF profile
LAST_RESULTS = None         # BassKernelResults of the last run (for test.py)

_CACHE = {}


def _build_program():
    if "nc" in _CACHE:
        return _CACHE["nc"]
    import concourse.bacc as bacc
    import concourse.tile as tile
    from concourse import mybir

    nc = bacc.Bacc("TRN2", target_bir_lowering=False, debug=False,
                   enable_asserts=False)
    bf = mybir.dt.bfloat16
    f32 = mybir.dt.float32

    wdev = nc.dram_tensor("wdev", [128, L, NCOL], bf, kind="ExternalInput")
    # cpack = [lhs_f | lhs_g | initf | initg] in one DMA-able constant
    cpack = nc.dram_tensor("cpack", [128, 256 + 2 * NCOL], bf,
                           kind="ExternalInput")
    terms = nc.dram_tensor("terms", [2, 128, TERMS_F], f32,
                           kind="ExternalInput")
    out_f = nc.dram_tensor("out_f", [128, NCOL], bf, kind="ExternalOutput")
    out_g = nc.dram_tensor("out_g", [128, NCOL], bf, kind="ExternalOutput")
    out_score = nc.dram_tensor("out_score", [2, 128, 1], f32,
                               kind="ExternalOutput")

    with tile.TileContext(nc) as tc:
        with (
            tc.tile_pool(name="const", bufs=1) as constp,
            tc.tile_pool(name="wstream", bufs=1) as wp,
            tc.tile_pool(name="stF", bufs=3) as stFp,
            tc.tile_pool(name="stG", bufs=3) as stGp,
            tc.tile_pool(name="cpF", bufs=3) as cpFp,
            tc.tile_pool(name="cpG", bufs=3) as cpGp,
            tc.tile_pool(name="fin", bufs=1) as finp,
            tc.tile_pool(name="psF", bufs=3, space="PSUM") as psFp,
            tc.tile_pool(name="psG", bufs=3, space="PSUM") as psGp,
        ):
            # scan-critical consts: ONE sync-queue DMA
            cp_t = constp.tile([128, 256 + 2 * NCOL], bf)
            nc.sync.dma_start(out=cp_t[:], in_=cpack[:])
            lhs_f = cp_t[:, 0:128]
            lhs_g = cp_t[:, 128:256]
            inf_t = cp_t[:, 256:256 + NCOL]
            ing_t = cp_t[:, 256 + NCOL:256 + 2 * NCOL]

            # W pieces stay resident (written once, never reused), issued
            # over three DGE queues so descriptor generation parallelizes
            piece_t = {}
            for queue, plist in ((nc.sync, PIECES_SP), (nc.scalar, PIECES_ACT),
                                 (nc.gpsimd, PIECES_POOL)):
                for (a, b) in plist:
                    wt = wp.tile([128, b - a, NCOL], bf, tag=f"wt{a}")
                    queue.dma_start(out=wt[:], in_=wdev[:, a:b, :])
                    piece_t[(a, b)] = wt

            def wslab(s, c0, c1):
                for (a, b) in PIECES:
                    if a <= s < b:
                        return piece_t[(a, b)][:, s - a, c0:c1]
                raise AssertionError(s)

            # terms stream in early (read only after the scan by ACT)
            terms_t = []
            for ch in range(2):
                tt = constp.tile([128, TERMS_F], f32, tag=f"terms{ch}")
                nc.gpsimd.dma_start(out=tt[:], in_=terms[ch, :, :])
                terms_t.append(tt)

            stF, stG = inf_t, ing_t
            mult = mybir.AluOpType.mult
            copyf = mybir.ActivationFunctionType.Copy
            for s in range(L):
                for dir_, (stp_, psp, cpp, st) in enumerate((
                        (stFp, psFp, cpFp, stF), (stGp, psGp, cpGp, stG))):
                    lhs = lhs_f if dir_ == 0 else lhs_g
                    slab = s if dir_ == 0 else L - 1 - s
                    v = psp.tile([128, NCOL], f32, tag="v")
                    nc.tensor.matmul(out=v[:], lhsT=lhs, rhs=st[:],
                                     start=True, stop=True)
                    n_ = stp_.tile([128, NCOL], bf, tag="n")
                    # path A: DVE direct from PSUM (f32, 1 elem/cycle)
                    nc.vector.tensor_tensor(out=n_[:, 0:NA], in0=v[:, 0:NA],
                                            in1=wslab(slab, 0, NA), op=mult)
                    # path B: ACT copies PSUM->SBUF bf16; DVE (4x all-SBUF
                    # mode) and Pool multiply from the copy
                    cpy = cpp.tile([128, NCPY], bf, tag="cpy")
                    nc.scalar.activation(out=cpy[:], in_=v[:, NA:NCOL],
                                         func=copyf)
                    nc.vector.tensor_tensor(out=n_[:, NA:NA + NB],
                                            in0=cpy[:, 0:NB],
                                            in1=wslab(slab, NA, NA + NB),
                                            op=mult)
                    if NP > 0:
                        nc.gpsimd.tensor_tensor(out=n_[:, NA + NB:NCOL],
                                                in0=cpy[:, NB:NCPY],
                                                in1=wslab(slab, NA + NB, NCOL),
                                                op=mult)
                    if dir_ == 0:
                        stF = n_
                    else:
                        stG = n_

            nc.sync.dma_start(out=out_f[:], in_=stF[:])
            nc.sync.dma_start(out=out_g[:], in_=stG[:])

            # gold-path score: ACT reduces the terms after its scan copies
            dump = constp.tile([128, TERMS_F], f32, tag="dump")
            for ch in range(2):
                sc = finp.tile([128, 1], f32, tag=f"sc{ch}")
                nc.scalar.activation(out=dump[:], in_=terms_t[ch][:],
                                     func=copyf, accum_out=sc[:])
                nc.scalar.dma_start(out=out_score[ch, :, :], in_=sc[:])

    nc.compile()
    _CACHE["nc"] = nc
    return nc


def _calibrate_C(logits, lens_, M, E):
    """Mean per-step growth of the scaled forward recursion, estimated on a
    small subsample.  C only conditions dynamic range, never correctness."""
    bs = np.arange(0, B, max(1, B // 128))
    E64 = E.astype(np.float64)
    lg = logits[bs].astype(np.float64)
    Ms = M[bs].astype(np.float64)
    lv = lens_[bs]
    up = np.zeros((K, len(bs))); up[START_IDX] = 1.0
    grs = []
    for t in range(T // 2):
        up = (E64.T @ up) * np.exp(lg[:, t, :] - Ms[:, t, None]).T
        m = up.max(axis=0)
        live = t < lv
        if live.any():
            grs.append(np.log(m[live]))
        up /= m
        up[:, ~live] = 0.0
        up[START_IDX, ~live] = 1.0
    return float(np.concatenate(grs).mean())


def kernel(logits, y_ent, lens, transitions):
    logits = np.ascontiguousarray(np.asarray(logits), dtype=F32)
    y = np.asarray(y_ent).astype(np.int64)
    lens_ = np.asarray(lens).astype(np.int64)
    trans = np.asarray(transitions).astype(F32)
    assert logits.shape == (B, T, K)

    # ---------------- host preprocessing ----------------
    Tc = np.maximum(trans, F32(-CLIP))
    E = np.exp(Tc.astype(np.float64)).astype(F32)
    E_bf = E.astype(BF16)
    M = logits.max(axis=2)                      # [B, T]
    Cconst = _calibrate_C(logits, lens_, M, E)

    # scaled emissions W[t, j, b] in bf16 with the pad/BOOST trick
    Wb = np.empty((T, K, B), dtype=BF16)
    pad_TB = np.arange(T)[:, None] >= lens_[None, :]          # [T, B]
    for t0 in range(0, T, 32):
        te = t0 + 32
        w = np.exp(logits[:, t0:te, :] - M[:, t0:te, None] - F32(Cconst))
        w = w.transpose(1, 2, 0)                              # [32, K, B]
        pm = pad_TB[t0:te]
        w = np.where(pm[:, None, :], F32(0.0), w)
        w[:, END_IDX, :] = np.where(pm, F32(BOOST), w[:, END_IDX, :])
        Wb[t0:te] = w.astype(BF16)

    # pack per-core W stream: slab[s][32q+j][c*64+bcol] =
    #   W[t=c*L+s, j, b=core*256+q*64+bcol]
    A = Wb.reshape(C, L, K, NCORES, 4, 64)
    A = np.ascontiguousarray(A.transpose(3, 4, 2, 1, 0, 5))
    wdev_np = A.reshape(NCORES, 128, L, NCOL)

    # cpack: cols 0:128 blockdiag(E) x4 (fwd lhsT: out = E^T u), 128:256
    # blockdiag(E^T) x4 (bwd), then initf / initg state tiles.
    # init states: fwd chunk0 = e_START, else ones; bwd chunk7 = BOOST*e_END,
    # else ones  (col x = c*64+bcol, partition 32q+j)
    cpack_np = np.zeros((128, 256 + 2 * NCOL), dtype=BF16)
    for q in range(4):
        cpack_np[32 * q:32 * q + 32, 32 * q:32 * q + 32] = E_bf
        cpack_np[32 * q:32 * q + 32, 128 + 32 * q:128 + 32 * q + 32] = E_bf.T
    initf_np = np.ones((128, NCOL), dtype=BF16)
    initg_np = np.ones((128, NCOL), dtype=BF16)
    colc = np.arange(NCOL) // 64                 # chunk of each column
    pj = np.arange(128) % 32                     # tag of each partition
    initf_np[:, colc == 0] = 0.0
    initf_np[np.ix_(pj == START_IDX, colc == 0)] = 1.0
    initg_np[:, colc == C - 1] = 0.0
    initg_np[np.ix_(pj == END_IDX, colc == C - 1)] = BF16(BOOST)
    cpack_np[:, 256:256 + NCOL] = initf_np
    cpack_np[:, 256 + NCOL:] = initg_np

    # gold-path score terms (host gathers + masks; device sums)
    e_scr = np.take_along_axis(logits, y[:, :, None], axis=2)[:, :, 0]
    e_terms = np.where(np.arange(T)[None, :] < lens_[:, None],
                       e_scr, F32(0.0)).astype(F32)            # [B, 512]
    labels_ext = np.concatenate(
        [np.full((B, 1), START_IDX, np.int64), y,
         np.full((B, 1), END_IDX, np.int64)], axis=1)
    pos = np.arange(T + 2)[None, :]
    labels_ext = np.where(pos < (lens_ + 1)[:, None], labels_ext, END_IDX)
    trn_scr = trans[labels_ext[:, :-1], labels_ext[:, 1:]]
    t_terms = np.where(np.arange(T + 1)[None, :] < (lens_ + 1)[:, None],
                       trn_scr, F32(0.0)).astype(F32)          # [B, 513]
    terms_np = np.zeros((NCORES, 2, 128, TERMS_F), dtype=F32)
    terms_np[..., 0:T] = e_terms.reshape(NCORES, 2, 128, T)
    terms_np[..., T:2 * T + 1] = t_terms.reshape(NCORES, 2, 128, T + 1)

    # per-sequence constant: sum_{t<len}(M + C)
    emask = np.arange(T)[None, :] < lens_[:, None]
    HC = ((M.astype(np.float64) * emask).sum(axis=1) + Cconst * lens_)

    # ---------------- run on the 8 cores ----------------
    nc = _build_program()
    from concourse.bass_utils import run_bass_kernel_spmd

    in_maps = [
        dict(wdev=wdev_np[core], cpack=cpack_np, terms=terms_np[core])
        for core in range(NCORES)
    ]
    res = run_bass_kernel_spmd(nc, in_maps, core_ids=list(range(NCORES)),
                               trace=TRACE)
    global LAST_RESULTS
    LAST_RESULTS = res

    # ---------------- host combine (f64) ----------------
    E64 = E_bf.astype(np.float64)
    logZ = np.zeros(B, np.float64)
    score = np.zeros(B, np.float64)
    for core in range(NCORES):
        r = res.results[core]
        ftile = r["out_f"].astype(np.float64)      # [128, 512]
        gtile = r["out_g"].astype(np.float64)
        sc = r["out_score"].reshape(-1).astype(np.float64)   # [256]
        # unpack [128=(q,j), 512=(c,bcol)] -> [C, K, 256=(q,bcol)]
        f = ftile.reshape(4, K, C, 64).transpose(2, 1, 0, 3).reshape(C, K, BS)
        z0 = gtile.reshape(4, K, C, 64).transpose(2, 1, 0, 3).reshape(C, K, BS)
        lz = np.zeros(BS, np.float64)
        ETf = np.einsum('ij,cib->cjb', E64, f)     # E^T @ f_c
        for c in range(C - 1):
            lz -= np.log(ETf[c].sum(axis=0))       # s_c = ones^T E^T f_c
        g = np.einsum('ij,cjb->cib', E64, z0)      # g_c = E z0_c
        for c in range(1, C):
            lz += np.log((g[c] * f[c - 1]).sum(axis=0))
        lz += np.log(g[0][START_IDX])              # g_0^T u0
        sl = slice(core * BS, (core + 1) * BS)
        logZ[sl] = lz
        score[sl] = sc

    nll = logZ + HC - 32.0 * math.log(2.0) - score
    return nll.astype(F32)
